# revision 16
# baseline (speedup 1.0000x reference)
"""Causal self-attention kernel for 8 Trainium2 NeuronCores.

Problem: B=4, T=2048, C=1024, H=16 heads, D=64 (fp32).
  qkv = x @ w_qkv + b_qkv ; causal softmax attention ; y @ w_proj + b_proj

Sharding: DP over batch (4) x TP over heads (2) = 8 cores.
Core c handles batch b=c//2 and heads h0=(c%2)*8 .. h0+7.
Each core computes a partial projection output (its 8 heads' contribution);
the host sums the two TP partials per batch and adds b_proj.

Precision plan (fp8e4m3 DoubleRow matmuls run at 0.5 cyc/row; bf16/f32r at
1.0). Softmax noise averages out over a row's keys, so early rows (few keys)
need precision and late rows don't:
  - q-group 0 (rows 0:512): bf16 qk matmul, bf16 qT/kT, bf16 exp output,
    bf16 AV against a bf16 copy of chunk-0 v.
  - q-groups 1-3: fp8 everywhere. Scores: fp8 DR qk matmul -> fp8 requant.
    ST via DR with a zeroed second pair half (contraction d=64). PT = fp8
    exp output. AV via DR with v split hi/lo fp8 (v = v_hi + v_lo keeps v
    error ~0.1%; v errors pass straight through to the output).
  - v16 = x(16Wv) via 3 compensated fp8 DR slots per C-tile:
    xh*wvh + xl*wvh + (x/32)*(32*(16Wv - wvh)).
  - 16x weight prescale (host) dodges fp8 subnormals; exp scale 2^-11 folds
    the prescale^2 and 1/sqrt(D) back out. proj in f32r with w_proj/16.
  - causal mask: only the 128-col diagonal wedge of each diagonal tile is
    masked (gpsimd.affine_select fill=0 on PT); ones col 64 of v_hi yields
    the softmax denominator row in the same AV matmul.

Schedule: ACT (exp) is the bottleneck engine (~146us busy), and PE executes
strictly in order, so emission order is software-pipelined:
  - Within attention, ST(j+2) is emitted after AV(j): ACT always has the
    next score tile ready while AV(j) blocks on exp(j).
  - Phase 1 of chunk n+1 and proj of chunk n-1 are emitted as small filler
    units inside attention(n)'s stream, soaking up PE stall time.
  - At the rep boundary, chunk-0 k/v filler units are placed after the last
    attention item that reads chunk-0 kT/v_aug (write-after-read).
"""

import numpy as np

B, T, C = 4, 2048, 1024
H, D = 16, 64
NCORES = 8
HC = H // 2  # heads per core (TP=2)
CEXP = 1.25  # softmax offset centering exp outputs in fp8e4m3 normal range
SCALE_EXP = 0.125 / 256.0  # 1/sqrt(D) * (16x weight prescale)^-2, exact 2^-11

TN = 512  # token chunk
NCHUNK = T // TN  # 4
KT_C = 8  # contraction tiles of 128 for C
KP_C = 4  # DoubleRow pairs for C
NVSUB = TN // 128  # 4 v sub-tiles per chunk
NKT = T // 128  # 16 k-token tiles
KT_P = (HC * D) // 128  # 4 contraction tiles for proj (512 feats)

_CACHE = {}


def _build_program(reps=1):
    # reps>1 repeats the whole kernel body inside one program (timing only:
    # the slope between rep counts isolates HW exec time from RPC overhead).
    import concourse.mybir as mybir
    import concourse.tile as tile
    from concourse import bacc

    f32 = mybir.dt.float32
    f32r = mybir.dt.float32r
    f8 = mybir.dt.float8e4
    bf16 = mybir.dt.bfloat16
    DR = mybir.MatmulPerfMode.DoubleRow
    Exp = mybir.ActivationFunctionType.Exp

    nc = bacc.Bacc("TRN2", target_bir_lowering=False, debug=False)

    xp = nc.dram_tensor("xp", [C, 3, T], f8, kind="ExternalInput").ap()
    xb = nc.dram_tensor("xb", [C, TN], bf16, kind="ExternalInput").ap()
    wqk = nc.dram_tensor("wqk", [C, C], f8, kind="ExternalInput").ap()
    wqkb = nc.dram_tensor("wqkb", [C, C], bf16, kind="ExternalInput").ap()
    wv = nc.dram_tensor("wv", [C, 2, HC * D], f8, kind="ExternalInput").ap()
    wproj = nc.dram_tensor("wproj", [HC * D, C], f32, kind="ExternalInput").ap()
    bqk = nc.dram_tensor("bqk", [C], f32, kind="ExternalInput").ap()
    bv = nc.dram_tensor("bv", [HC * D], f32, kind="ExternalInput").ap()
    out = nc.dram_tensor("out", [T, C], f32, kind="ExternalOutput").ap()

    xp_r = xp.rearrange("(kt p) v t -> p kt v t", p=128)  # [128, 8, 3, 2048]
    xb_r = xb.rearrange("(kt p) t -> p kt t", p=128)  # [128, 8, 512]
    wqk_r = wqk.rearrange("(kp two p) f -> p kp two f", p=128, two=2)
    wqkb_r = wqkb.rearrange("(kt p) f -> p kt f", p=128)  # [128, 8, 1024]
    wv_r = wv.rearrange("(kt p) two f -> p kt two f", p=128)  # [128, 8, 2, 512]
    wproj_r = wproj.rearrange("(ko p) f -> p ko f", p=128)  # [128, 4, 1024]
    bqk_r = bqk.rearrange("(m p) -> p m", p=128)  # [128, 8]

    tc_ctx = tile.TileContext(nc)
    tc = tc_ctx.__enter__()
    pools = [
        tc.tile_pool(name="pers", bufs=1),
        tc.tile_pool(name="xc", bufs=2),
        tc.tile_pool(name="qtc", bufs=2),
        tc.tile_pool(name="ptp", bufs=4),
        tc.tile_pool(name="ptb", bufs=4),
        tc.tile_pool(name="vfp", bufs=2),
        tc.tile_pool(name="otc", bufs=2),
        tc.tile_pool(name="outp", bufs=3),
        tc.tile_pool(name="rcp", bufs=2),
        tc.tile_pool(name="rcbp", bufs=2),
        tc.tile_pool(name="ps_qv", bufs=2, space="PSUM"),
        tc.tile_pool(name="ps_st", bufs=2, space="PSUM"),
        tc.tile_pool(name="ps_ot", bufs=2, space="PSUM"),
    ]
    (pers, xcp, qtcp, ptp, ptb, vfp, otcp, outp, rcp, rcbp,
     ps_qv, ps_st, ps_ot) = [p.__enter__() for p in pools]

    # --- persistent tiles ---
    wqk_sb = pers.tile([128, KP_C, 2, C], f8)  # [128,4,2,1024]
    wqkb_sb = pers.tile([128, KT_C, C], bf16)  # [128, 8, 1024]
    wv_sb = pers.tile([128, KT_C, 2, HC * D], f8)  # [128, 8, 2, 512]
    wpj_sb = pers.tile([128, KT_P, C], f32r)  # [128, 4, 1024]
    kT_sb = pers.tile([128, KT_P, 2, T], f8)  # [128, 4, 2, 2048]
    kT0b = pers.tile([128, KT_P, TN], bf16)  # chunk-0 kT, bf16
    qT0b = pers.tile([128, KT_P, TN], bf16)  # chunk-0 qT, bf16
    x0b = pers.tile([128, KT_C, TN], bf16)  # chunk-0 x, bf16
    # AV lhsT: cols 0:64 = v16 hi/lo, col 64 = ones (denominator row in
    # pair 0), cols 65:96 zero pad (DoubleRow output partitions must be
    # 32-aligned; 65 crashes walrus codegen)
    v_aug = pers.tile([128, NKT, 2, HC, 96], f8)
    v0b = pers.tile([128, NVSUB, HC, D + 1], bf16)  # chunk-0 v16, bf16
    bqk_sb = pers.tile([128, KT_C], f32)
    bv_bc = pers.tile([128, HC * D], f32)
    bv_row = pers.tile([1, HC * D], f32)
    neg_c = pers.tile([128, 1], f32)

    nc.vector.memset(neg_c[:], -CEXP)
    nc.vector.memset(kT_sb[:, :, 1, :], 0.0)  # zero second ST pair half
    nc.vector.memset(v_aug[:, :, 0], 0.0)
    nc.vector.memset(v_aug[:, :, 1], 0.0)
    nc.vector.memset(v_aug[:, :, 0, :, D : D + 1], 1.0)  # denominator ones
    nc.vector.memset(v0b[:, :, :, D : D + 1], 1.0)
    nc.sync.dma_start(bqk_sb[:], bqk_r)
    nc.sync.dma_start(bv_row[:], bv[None, :])
    nc.gpsimd.partition_broadcast(bv_bc[:], bv_row[:])

    def dma_x0b():
        nc.sync.dma_start(x0b[:], xb_r)

    def dma_xc(xc, n, part):
        sl = slice(n * TN, (n + 1) * TN)
        if part == 0:  # variant 0 (xh) feeds qk
            nc.sync.dma_start(xc[:, 0:4, 0, :], xp_r[:, 0:4, 0, sl])
            nc.sync.dma_start(xc[:, 4:8, 0, :], xp_r[:, 4:8, 0, sl])
        else:  # variants 1,2 (x/32, xl) feed v
            nc.sync.dma_start(xc[:, :, 1, :], xp_r[:, :, 1, sl])
            nc.sync.dma_start(xc[:, :, 2, :], xp_r[:, :, 2, sl])

    def qk_unit_fp8(xc, qTc, n, m):
        """One qk m-tile for chunk n>=1: 4 DR matmuls + fp8 quantize-copy."""
        ps = ps_qv.tile([128, TN], f32, tag="qv")
        for kp in range(KP_C):
            nc.tensor.matmul(
                ps[:],
                wqk_sb[:, kp, :, m * 128 : (m + 1) * 128],
                xc[:, 2 * kp : 2 * kp + 2, 0, :],
                start=(kp == 0),
                stop=(kp == KP_C - 1),
                perf_mode=DR,
            )
        if m < 4:
            dst = qTc[:, m, :]
        else:
            dst = kT_sb[:, m - 4, 0, n * TN : (n + 1) * TN]
        nc.vector.tensor_scalar_add(dst, ps[:], bqk_sb[:, m : m + 1])

    def qk_unit_bf16(m):
        """One qk m-tile for chunk 0: bf16 matmuls; k also written as fp8."""
        ps = ps_qv.tile([128, TN], f32, tag="qv")
        for kt in range(KT_C):
            nc.tensor.matmul(
                ps[:],
                wqkb_sb[:, kt, m * 128 : (m + 1) * 128],
                x0b[:, kt, :],
                start=(kt == 0),
                stop=(kt == KT_C - 1),
            )
        if m < 4:
            nc.vector.tensor_scalar_add(qT0b[:, m, :], ps[:], bqk_sb[:, m : m + 1])
        else:
            nc.vector.tensor_scalar_add(kT0b[:, m - 4, :], ps[:], bqk_sb[:, m : m + 1])
            nc.vector.tensor_scalar_add(
                kT_sb[:, m - 4, 0, 0:TN], ps[:], bqk_sb[:, m : m + 1]
            )

    def v_unit(xc, n, mm):
        """v16 for one 128-token subtile: 12 DR matmuls + hi/lo split."""
        ktg = n * NVSUB + mm
        sub = slice(mm * 128, (mm + 1) * 128)
        psv = ps_qv.tile([128, HC * D], f32, tag="qv")
        for kt in range(KT_C):  # xh*wvh + (x/32)*wvl
            nc.tensor.matmul(
                psv[:],
                xc[:, kt, 0:2, sub],
                wv_sb[:, kt, :, :],
                start=(kt == 0),
                stop=False,
                perf_mode=DR,
            )
        for kp in range(KP_C):  # + xl*wvh
            nc.tensor.matmul(
                psv[:],
                xc[:, 2 * kp : 2 * kp + 2, 2, sub],
                wv_sb[:, 2 * kp : 2 * kp + 2, 0, :],
                start=False,
                stop=(kp == KP_C - 1),
                perf_mode=DR,
            )
        vf = vfp.tile([128, HC * D], f32)
        nc.vector.tensor_add(vf[:], psv[:], bv_bc[:])
        vf_hd = vf[:].rearrange("p (h d) -> p h d", d=D)
        nc.gpsimd.tensor_copy(v_aug[:, ktg, 0, :, 0:D], vf_hd)
        nc.vector.tensor_sub(
            v_aug[:, ktg, 1, :, 0:D], vf_hd, v_aug[:, ktg, 0, :, 0:D]
        )
        if n == 0:
            nc.gpsimd.tensor_copy(v0b[:, mm, :, 0:D], vf_hd)

    def proj_unit(otc_prev, qg_prev, mm, nn):
        """proj output block [128 tokens, 512 feats] in f32r."""
        pp = ps_qv.tile([128, TN], f32, tag="qv")
        for kt in range(KT_P):
            nc.tensor.matmul(
                pp[:],
                otc_prev[:, kt, mm * 128 : (mm + 1) * 128],
                wpj_sb[:, kt, nn * TN : (nn + 1) * TN],
                start=(kt == 0),
                stop=(kt == KT_P - 1),
            )
        ob = outp.tile([128, TN], f32)
        nc.vector.tensor_copy(ob[:], pp[:])
        r0 = qg_prev * TN + mm * 128
        out_dmas.append(
            lambda ob=ob, r0=r0, nn=nn: nc.sync.dma_start(
                out[r0 : r0 + 128, nn * TN : (nn + 1) * TN], ob[:]
            )
        )

    def emit_attention(n, qTc, otc, fillers):
        """Attention for q-group n, lag-1 ST->AV pipeline, fillers mixed in.

        fillers: list of (pos, fn) emitted once item index pos is reached.
        """
        qg = n
        bf = qg == 0
        kt_max = NVSUB * (qg + 1)
        npairs = kt_max // 2
        items = [(h, pa) for h in range(HC) for pa in range(npairs)]
        state = {}
        psos = {}

        def st_width(kt):
            j = kt - NVSUB * qg
            return TN if j < 0 else TN - 128 * j

        def emit_ST(j):
            h, pa = items[j]
            pb, ko = (h % 2) * 64, h // 2
            ka, kb = 2 * pa, 2 * pa + 1
            wa, wb = st_width(ka), st_width(kb)
            pss = ps_st.tile([128, 2 * TN], f32, tag="pss")
            for kt, off, w in ((ka, 0, wa), (kb, wa, wb)):
                if bf:
                    nc.tensor.matmul(
                        pss[:, off : off + w],
                        kT0b[pb : pb + 64, ko, kt * 128 : (kt + 1) * 128],
                        qT0b[pb : pb + 64, ko, TN - w : TN],
                        start=True,
                        stop=True,
                    )
                else:
                    qs = qTc[pb : pb + 64, ko, TN - w : TN]
                    nc.tensor.matmul(
                        pss[:, off : off + w],
                        kT_sb[pb : pb + 64, ko, :, kt * 128 : (kt + 1) * 128],
                        qs.unsqueeze(1).broadcast_to((64, 2, w)),
                        start=True,
                        stop=True,
                        perf_mode=DR,
                    )
            pt = (ptb if bf else ptp).tile([128, 2 * TN], bf16 if bf else f8)
            nc.scalar.activation(
                pt[:, 0 : wa + wb], pss[:, 0 : wa + wb], Exp,
                bias=neg_c[:], scale=SCALE_EXP,
            )
            if ka >= NVSUB * qg:  # diagonal pair: 128-col wedge masks
                for off in (0, wa):
                    nc.gpsimd.affine_select(
                        out=pt[:, off : off + 128],
                        in_=pt[:, off : off + 128],
                        compare_op=mybir.AluOpType.is_ge,
                        fill=0.0,
                        base=0,
                        pattern=[[1, 128]],
                        channel_multiplier=-1,
                    )
            state[j] = (pt, wa, wb)

        def emit_AV(j):
            h, pa = items[j]
            pb, ko = (h % 2) * 64, h // 2
            if pa == 0:
                psos[h] = ps_ot.tile([96, TN], f32, name="pso", tag="pso")
            pso = psos[h]
            pt, wa, wb = state.pop(j)
            ka, kb = 2 * pa, 2 * pa + 1
            for kt, off, w in ((ka, 0, wa), (kb, wa, wb)):
                if bf:
                    nc.tensor.matmul(
                        pso[0 : D + 1, TN - w : TN],
                        v0b[:, kt, h, :],
                        pt[:, off : off + w],
                        start=(kt == 0),
                        stop=(kt == kt_max - 1),
                    )
                else:
                    pts = pt[:, off : off + w]
                    nc.tensor.matmul(
                        pso[:, TN - w : TN],
                        v_aug[:, kt, :, h, :],
                        pts.unsqueeze(1).broadcast_to((128, 2, w)),
                        start=(kt == 0),
                        stop=(kt == kt_max - 1),
                        perf_mode=DR,
                    )
            if pa == npairs - 1:  # head done: normalize into otc
                rc = rcp.tile([1, TN], f32)
                nc.vector.reciprocal(rc[:], pso[D : D + 1, :])
                rcb = rcbp.tile([64, TN], f32)
                nc.gpsimd.partition_broadcast(rcb[:], rc[:])
                nc.vector.tensor_mul(
                    out=otc[pb : pb + 64, ko, :], in0=pso[0:D, :], in1=rcb[:]
                )

        fillers = sorted(fillers, key=lambda x: x[0])
        fi = 0
        emit_ST(0)
        if len(items) > 1:
            emit_ST(1)
        for j in range(len(items)):
            emit_AV(j)
            if j + 2 < len(items):
                emit_ST(j + 2)
            while fi < len(fillers) and fillers[fi][0] <= j:
                fillers[fi][1]()
                fi += 1
        while fi < len(fillers):
            fillers[fi][1]()
            fi += 1
        for fn in out_dmas:  # output stores: data long ready, zero SEQ wait
            fn()
        out_dmas.clear()

    def spread(n_items, units):
        """Evenly assign positions 0..n_items-1 to units."""
        U = len(units)
        return [
            (min(n_items - 1, (u * n_items) // U), fn)
            for u, fn in enumerate(units)
        ]

    # ---------------- program ----------------
    MO = (0, 4, 1, 5, 2, 6, 3, 7)  # q/k interleaved m-order
    out_dmas = []

    # prologue: chunk 0 of rep 0 (DMA order: feed the bf16 qk units first)
    dma_x0b()
    nc.sync.dma_start(wqkb_sb[:], wqkb_r)
    xc0 = xcp.tile([128, KT_C, 3, TN], f8, name="xc", tag="xc")
    dma_xc(xc0, 0, 0)
    dma_xc(xc0, 0, 1)
    nc.sync.dma_start(wv_sb[:], wv_r)
    for m in MO:
        qk_unit_bf16(m)
    nc.sync.dma_start(wqk_sb[:], wqk_r)
    nc.sync.dma_start(wpj_sb[:], wproj_r.bitcast(f32r))
    for mm in range(NVSUB):
        v_unit(xc0, 0, mm)

    otcs = {}
    xcs = {0: xc0}
    qtcs = {}
    for r in range(reps):
        last_rep = r == reps - 1
        for n in range(NCHUNK):
            otcs[n] = otcp.tile([128, KT_P, TN], f32r, name="otc", tag="otc")
            units = []
            late_units = []
            nn_ = n + 1
            if nn_ < NCHUNK:  # phase 1 of next chunk (same rep)
                xc = xcp.tile([128, KT_C, 3, TN], f8, name="xc", tag="xc")
                qtc2 = qtcp.tile([128, KT_P, TN], f8, name="qtc", tag="qtc")
                qtcs[nn_] = qtc2
                units.append(lambda xc=xc, p=0, n2=nn_: dma_xc(xc, n2, p))
                units.append(lambda xc=xc, p=1, n2=nn_: dma_xc(xc, n2, p))
                for m in MO:
                    units.append(
                        lambda xc=xc, q=qtc2, n2=nn_, m=m: qk_unit_fp8(xc, q, n2, m)
                    )
                for mm in range(NVSUB):
                    units.append(lambda xc=xc, n2=nn_, mm=mm: v_unit(xc, n2, mm))
            elif not last_rep:  # chunk 0 of the next rep (bf16 path)
                xc = xcp.tile([128, KT_C, 3, TN], f8, name="xc", tag="xc")
                xcs[0] = xc
                units.append(dma_x0b)
                units.append(lambda xc=xc: dma_xc(xc, 0, 0))
                units.append(lambda xc=xc: dma_xc(xc, 0, 1))
                for m in (0, 1, 2, 3):  # q units: no chunk-0 readers to wait on
                    units.append(lambda m=m: qk_unit_bf16(m))
                # k/v units overwrite chunk-0 kT/v_aug: place after the last
                # attention item that reads them (pair 0/1 of head 7)
                for m in (4, 5, 6, 7):
                    late_units.append(lambda m=m: qk_unit_bf16(m))
                for mm in range(NVSUB):
                    late_units.append(lambda xc=xc, mm=mm: v_unit(xc, 0, mm))
            # proj of the previous chunk (chunk 3 of the previous rep at n=0)
            qg_prev = n - 1 if n >= 1 else 3
            otc_prev = otcs.get(qg_prev) if (n >= 1 or r > 0) else None
            if otc_prev is not None:
                for mm in range(NVSUB):
                    for nnn in range(2):
                        units.append(
                            lambda o=otc_prev, q=qg_prev, mm=mm, nnn=nnn:
                            proj_unit(o, q, mm, nnn)
                        )
            n_items = HC * (NVSUB * (n + 1) // 2)
            fillers = spread(n_items, units)
            if late_units:
                base = HC * NVSUB * (n + 1) // 2 - 6
                fillers += [(base + u // 3, fn) for u, fn in enumerate(late_units)]
            emit_attention(n, qtcs.get(n), otcs[n], fillers)
        if last_rep:  # epilogue: proj of the final chunk
            for mm in range(NVSUB):
                for nnn in range(2):
                    proj_unit(otcs[3], 3, mm, nnn)
            for fn in out_dmas:
                fn()
            out_dmas.clear()

    for p in reversed(pools):
        p.__exit__(None, None, None)
    tc_ctx.__exit__(None, None, None)

    nc.compile()
    return nc


def _prep_inputs(x, w_qkv, b_qkv, w_proj):
    """Shard full inputs into 8 per-core input maps (fp8/bf16 quantized)."""
    import concourse.mybir as mybir

    F8 = mybir.dt.np(mybir.dt.float8e4)
    BF = mybir.dt.np(mybir.dt.bfloat16)
    x = np.asarray(x, dtype=np.float32)
    w_qkv = np.asarray(w_qkv, dtype=np.float32)
    b_qkv = np.asarray(b_qkv, dtype=np.float32)
    w_proj = np.asarray(w_proj, dtype=np.float32)

    Wq, Wk, Wv = w_qkv[:, :C], w_qkv[:, C : 2 * C], w_qkv[:, 2 * C :]
    bq, bk, bvv = b_qkv[:C], b_qkv[C : 2 * C], b_qkv[2 * C :]

    xps, xbs = [], []
    for b in range(B):
        xT = np.ascontiguousarray(x[b].T)  # [C, T]
        xh = xT.astype(F8)
        xh32 = (xT / 32.0).astype(F8)
        xl = (xT - xh.astype(np.float32)).astype(F8)
        xps.append(np.ascontiguousarray(np.stack([xh, xh32, xl], axis=1)))
        xbs.append(np.ascontiguousarray(xT[:, :TN].astype(BF)))

    in_maps = []
    for c in range(NCORES):
        b, t = divmod(c, 2)
        sl = slice(t * HC * D, (t + 1) * HC * D)
        wqk16 = np.concatenate([16.0 * Wq[:, sl], 16.0 * Wk[:, sl]], axis=1)
        Wv16 = 16.0 * Wv[:, sl]
        wvh = Wv16.astype(F8)
        wvl = (32.0 * (Wv16 - wvh.astype(np.float32))).astype(F8)
        in_maps.append(
            {
                "xp": xps[b],
                "xb": xbs[b],
                "wqk": np.ascontiguousarray(wqk16.astype(F8)),
                "wqkb": np.ascontiguousarray(wqk16.astype(BF)),
                "wv": np.ascontiguousarray(np.stack([wvh, wvl], axis=1)),
                "wproj": np.ascontiguousarray(w_proj[sl, :] / 16.0),
                "bqk": np.ascontiguousarray(
                    16.0 * np.concatenate([bq[sl], bk[sl]])
                ),
                "bv": np.ascontiguousarray(16.0 * bvv[sl]),
            }
        )
    return in_maps


def _run(x, w_qkv, b_qkv, w_proj, b_proj, trace=False, **trace_kwargs):
    from concourse.bass_utils import run_bass_kernel_spmd

    if "nc" not in _CACHE:
        _CACHE["nc"] = _build_program()
    nc = _CACHE["nc"]

    in_maps = _prep_inputs(x, w_qkv, b_qkv, w_proj)
    res = run_bass_kernel_spmd(
        nc, in_maps, list(range(NCORES)), trace=trace, **trace_kwargs
    )

    b_proj = np.asarray(b_proj, dtype=np.float32)
    y = np.empty((B, T, C), dtype=np.float32)
    for b in range(B):
        y[b] = res.results[2 * b]["out"] + res.results[2 * b + 1]["out"] + b_proj
    return y, res


def kernel(x, w_qkv, b_qkv, w_proj, b_proj):
    y, _ = _run(x, w_qkv, b_qkv, w_proj, b_proj, trace=False)
    return y
